# revision 1
# baseline (speedup 1.0000x reference)
"""Trainium2 Bass kernel for nn_CSTri (membrane / cloth triangle energy).

Math: the reference computes, per face, the eigenvalues of the 2x2
Cauchy-Green tensor C = F^T F built from an intrinsic 2D basis of the
reference triangle.  C is similar to G @ R^{-1} where G and R are the 2x2
edge Gram matrices of the deformed / reference triangle:

    G = [[|e0|^2, e0.e1], [e0.e1, |e1|^2]]   (deformed edges, per batch)
    R = same for reference edges              (per face, batch-independent)

so eig(C) = eig(G R^{-1}):  tr = (s00 r11 - 2 s01 r01 + s11 r00)/detR,
det = detG/detR.  All the cross products / normalisations in the reference
cancel, and f_rest_areas = sqrt(detR)/2.

Tension-field relaxation is handled branch-free: with
eig_max := max(t + rh, 1), emt := eig_max^{-1/2}, emin := max(t - rh, emt),
the energy-density-plus-mu  en0 = 0.5*mu*(eig_max+emin) + (lam/8*L - mu/2)*L
(L = ln(eig_max*emin)) equals exactly mu when the clamp engages
(eig_max' = 1 -> L = 0, emin = 1), so  energy_density = en0 - mu  is exactly
0 for compressed faces -- no mask needed.  The constant -mu is folded into
the final host-side reduction via sum(w).

Sharding: faces = arange(V).reshape(F, 3), so face f uses exactly vertices
3f, 3f+1, 3f+2 -- the "gather" is a reshape and an even split of the face
dim across 8 cores is a contiguous slice of the vertex dim.

Per core layout: [128 partitions, 512 faces] fp32 tiles; the raw 9 floats
per face stay interleaved in SBUF and are read with stride-9 access
patterns (free for fp32 1x DVE ops).
"""

import numpy as np

B, V, F, M = 8, 1572864, 524288, 8
FC = F // M            # 65536 faces per core
VC = V // M            # 196608 vertices per core
P, W = 128, 512        # FC = P * W
POISSON = 0.33
EPS = 1e-15
LN_HALF = -0.6931471805599453

LAST_RESULTS = None    # BassKernelResults of the most recent run (for test.py)


def _split_multi_waits(nc, mybir):
    """Walrus in this image caps sync waits at 1/instruction (2 for
    EventSemaphore); Tile can emit more.  Move extras onto NoOps."""
    for fn in nc.m.functions:
        for bb in fn.blocks:
            insts = bb.instructions
            new_list = []
            changed = False
            for inst in insts:
                si = inst.sync_info
                waits = list(si.on_wait) if si is not None and si.on_wait else []
                cap = 2 if inst.opcode == "EventSemaphore" else 1
                if len(waits) > cap:
                    extra, keep = waits[:-cap], waits[-cap:]
                    for k, w in enumerate(extra):
                        new_list.append(mybir.InstNoOp(
                            name=f"{inst.name}_wsplit{k}",
                            sync_info=mybir.SyncInfo(on_wait=[w], on_update=[]),
                            engine=inst.engine,
                            bass_nofuse=True,
                        ))
                    si.on_wait = keep
                    inst.sync_info = si
                    changed = True
                new_list.append(inst)
            if changed:
                insts[:] = new_list


def _build(mu, lam, waitsplit=True, bf16_tail=False):
    import concourse.bass as bass
    import concourse.mybir as mybir
    from concourse.tile import TileContext

    dt = mybir.dt.float32
    dtt = mybir.dt.bfloat16 if bf16_tail else dt
    Alu = mybir.AluOpType
    Act = mybir.ActivationFunctionType

    nc = bass.Bass()
    if bf16_tail:
        nc._allow_low_precision_reason = "bf16 energy tail; face sums accumulate in fp32 accum_out"
    verts = nc.declare_dram_parameter("verts", [B, VC, 3], dt, isOutput=False)
    vref = nc.declare_dram_parameter("vref", [VC, 3], dt, isOutput=False)
    thick = nc.declare_dram_parameter("thick", [FC], dt, isOutput=False)
    out = nc.declare_dram_parameter("out", [P, 16], dt, isOutput=True)

    with TileContext(nc) as tc:
        with (
            tc.tile_pool(name="xp", bufs=2) as xp,
            tc.tile_pool(name="coef", bufs=1) as coef,
            tc.tile_pool(name="sc", bufs=1) as sc,
        ):
            def T(tag, d=dt):
                return sc.tile([P, W], d, tag=tag, name=tag)

            def edges_and_gram(Xtile, pfx, sdt=dt):
                """Xtile: [P, 9W] interleaved verts -> (s00, s01, s11).

                Blocked layout: one strided sub produces e_int [P,(a=2,w,c=3)]
                (reads are 3-contiguous runs), one ACT square, one dense mul
                for e0*e1, then tensor_reduce over the innermost c=3.
                """
                Xq = Xtile.rearrange("p (w v c) -> p v w c", v=3, c=3)
                e_int = sc.tile([P, 6 * W], dt, tag=f"{pfx}ei", name=f"{pfx}ei")
                ev = e_int.rearrange("p (a w c) -> p a w c", a=2, c=3)
                v0 = Xq[:, 0, :, :]
                v0b = bass.AP(tensor=v0.tensor, offset=v0.offset,
                              ap=[v0.ap[0], [0, 2]] + list(v0.ap[1:]))
                nc.vector.tensor_sub(ev, Xq[:, 1:3, :, :], v0b)

                q_int = sc.tile([P, 6 * W], dt, tag=f"{pfx}qi", name=f"{pfx}qi")
                nc.scalar.activation(q_int, e_int, Act.Square)
                qv = q_int.rearrange("p (a w c) -> p a w c", a=2, c=3)

                m_int = sc.tile([P, 3 * W], dt, tag=f"{pfx}mi", name=f"{pfx}mi")
                mv = m_int.rearrange("p (w c) -> p w c", c=3)
                nc.vector.tensor_mul(mv, ev[:, 0], ev[:, 1])

                s3 = sc.tile([P, 3 * W], sdt, tag=f"{pfx}s3", name=f"{pfx}s3")
                s3v = s3.rearrange("p (k w) -> p k w", k=3)
                nc.vector.tensor_reduce(s3v[:, 0:2], qv, mybir.AxisListType.X, Alu.add)
                nc.vector.tensor_reduce(s3v[:, 2], mv, mybir.AxisListType.X, Alu.add)
                return s3

            # ---------------- per-face reference coefficients ----------------
            Rt = coef.tile([P, 9 * W], dt, name="Rt")
            nc.sync.dma_start(out=Rt, in_=vref.rearrange("(p w) c -> p (w c)", p=P))
            TH = coef.tile([P, W], dt, name="TH")
            nc.sync.dma_start(out=TH, in_=thick.rearrange("(p w) -> p w", p=P))

            b_lnh = coef.tile([P, 1], dt, name="b_lnh")
            nc.vector.memset(b_lnh, LN_HALF)
            b_t1 = coef.tile([P, 1], dt, name="b_t1")
            nc.vector.memset(b_t1, -0.5 * mu)


            # ---------------- per-batch face energies ----------------
            for b in range(B):
                X = xp.tile([P, 9 * W], dt, tag="X", name="X")
                nc.sync.dma_start(
                    out=X, in_=verts[b].rearrange("(p w) c -> p (w c)", p=P)
                )
                s3 = edges_and_gram(X, "b", sdt=dtt)
                if b == 0:
                    # Emit the per-face reference coefficients here: DVE chews
                    # on batch-0's Gram while ACT squares the ref edges, instead
                    # of stalling on the ref DMA at kernel start.
                    rs3 = edges_and_gram(Rt, "r")
                    rv = rs3.rearrange("p (k w) -> p k w", k=3)
                    r00, r11, r01 = rv[:, 0], rv[:, 1], rv[:, 2]
                    z = T("rz")
                    nc.vector.tensor_mul(z, r00, r11)
                    zz = T("rzz")
                    nc.scalar.activation(zz, r01, Act.Square)
                    detR = T("detR")
                    nc.vector.tensor_sub(detR, z, zz)
                    rec = T("rrec")
                    nc.vector.reciprocal(rec, detR)

                    P3 = coef.tile([P, 3 * W], dtt, tag="P3", name="P3")
                    P3v = P3.rearrange("p (k w) -> p k w", k=3)
                    qc = coef.tile([P, W], dtt, tag="qc", name="qc")
                    Wf = coef.tile([P, W], dt, tag="c4", name="c4")
                    # planes: (r11, r00, -2 r01)/(2 detR)  to pair with s3=(s00,s11,s01)
                    nc.vector.scalar_tensor_tensor(P3v[:, 0], r11, 0.5, rec, Alu.mult, Alu.mult)
                    nc.vector.scalar_tensor_tensor(P3v[:, 1], r00, 0.5, rec, Alu.mult, Alu.mult)
                    nc.vector.scalar_tensor_tensor(P3v[:, 2], r01, -1.0, rec, Alu.mult, Alu.mult)
                    nc.vector.tensor_scalar_mul(qc, rec, 0.25)
                    # Wf = 0.5*sqrt(detR)*thickness   (sqrt via exp(0.5 ln + ln 0.5))
                    ld = T("rld")
                    nc.scalar.activation(ld, detR, Act.Ln)
                    ex = T("rex")
                    nc.scalar.activation(ex, ld, Act.Exp, bias=b_lnh, scale=0.5)
                    nc.vector.tensor_mul(Wf, ex, TH)

                    out_t = coef.tile([P, 16], dt, name="out_t")
                    nc.vector.memset(out_t, 0.0)
                    nc.vector.tensor_reduce(out_t[:, 8:9], Wf, mybir.AxisListType.X, Alu.add)
                sv = s3.rearrange("p (k w) -> p k w", k=3)
                s00, s11, s01 = sv[:, 0], sv[:, 1], sv[:, 2]

                # t = tr/2 = sum_k s3[k] * P3[k]   (one mul + two adds)
                tm = sc.tile([P, 3 * W], dtt, tag="tm", name="tm")
                nc.vector.tensor_mul(tm, s3, P3)
                tmv = tm.rearrange("p (k w) -> p k w", k=3)
                ta = T("ta", dtt)
                nc.vector.tensor_add(ta, tmv[:, 0], tmv[:, 1])
                t = T("t", dtt)
                nc.vector.tensor_add(t, ta, tmv[:, 2])

                # d4 = det/4 = (s00 s11 - s01^2) * q
                z2 = T("z2", dtt)
                nc.vector.tensor_mul(z2, s00, s11)
                z1 = T("z1", dtt)
                nc.scalar.activation(z1, s01, Act.Square)
                nc.vector.tensor_sub(z2, z2, z1)
                d4 = T("d4", dtt)
                nc.vector.tensor_mul(d4, z2, qc)

                # rh = sqrt(max(t^2 - d4, EPS))
                u = T("u", dtt)
                nc.scalar.activation(u, t, Act.Square)
                ap_ = T("ap", dtt)
                nc.vector.tensor_sub(ap_, u, d4)
                nc.vector.tensor_scalar_max(ap_, ap_, EPS)
                la = T("la", dtt)
                nc.scalar.activation(la, ap_, Act.Ln)
                rh = T("rh", dtt)
                nc.scalar.activation(rh, la, Act.Exp, scale=0.5)

                emin = T("emin", dtt)
                nc.vector.tensor_sub(emin, t, rh)          # eig_min
                emax = T("emax", dtt)
                nc.vector.tensor_add(emax, t, rh)
                nc.vector.tensor_scalar_max(emax, emax, 1.0)  # relaxation clamp

                lm = T("lm", dtt)
                nc.scalar.activation(lm, emax, Act.Ln)
                emt = T("emt", dtt)
                nc.scalar.activation(emt, lm, Act.Exp, scale=-0.5)  # emax^-1/2
                nc.vector.tensor_max(emin, emin, emt)

                iic = T("iic", dtt)
                nc.vector.tensor_mul(iic, emax, emin)
                L = T("L", dtt)
                nc.scalar.activation(L, iic, Act.Ln)
                t1 = T("t1", dtt)
                nc.scalar.activation(t1, L, Act.Identity,
                                     bias=b_t1, scale=0.125 * lam)
                t2 = T("t2", dtt)
                nc.vector.tensor_mul(t2, t1, L)
                sum1 = T("sum1", dtt)
                nc.vector.tensor_add(sum1, emax, emin)
                en0 = T("en0", dtt)
                nc.vector.scalar_tensor_tensor(en0, sum1, 0.5 * mu, t2,
                                               Alu.mult, Alu.add)
                enw = T("enw", dtt)
                nc.vector.scalar_tensor_tensor(
                    enw, en0, 1.0, Wf, Alu.mult, Alu.mult,
                    accum_out=out_t[:, b:b + 1],
                )

            nc.sync.dma_start(out=out[:, :], in_=out_t)

    if waitsplit:
        _split_multi_waits(nc, mybir)
    return nc


def kernel(vertices, vertices_ref, faces, youngmoduli, thicknesses):
    import os
    from concourse.bass_utils import run_bass_kernel_spmd

    vertices = np.asarray(vertices)
    vertices_ref = np.asarray(vertices_ref)
    faces = np.asarray(faces)
    thicknesses = np.asarray(thicknesses)
    assert vertices.shape == (B, V, 3) and vertices_ref.shape == (V, 3)
    assert faces.shape == (F, 3)
    if not np.array_equal(faces, np.arange(V, dtype=faces.dtype).reshape(F, 3)):
        raise NotImplementedError("kernel assumes faces == arange(V).reshape(F,3)")

    ym = float(np.asarray(youngmoduli).reshape(-1)[0])
    mu = ym / (2.0 * (1.0 + POISSON))
    lam = ym * POISSON / ((1.0 + POISSON) * (1.0 - 2.0 * POISSON))

    import os as _os
    bf16_tail = _os.environ.get("KERNEL_BF16", "0") == "1"
    nc = _build(mu, lam, bf16_tail=bf16_tail)

    in_maps = []
    for m in range(M):
        in_maps.append({
            "verts": np.ascontiguousarray(
                vertices[:, m * VC:(m + 1) * VC, :], dtype=np.float32),
            "vref": np.ascontiguousarray(
                vertices_ref[m * VC:(m + 1) * VC, :], dtype=np.float32),
            "thick": np.ascontiguousarray(
                thicknesses[m * FC:(m + 1) * FC], dtype=np.float32),
        })

    trace = os.environ.get("KERNEL_TRACE", "0") == "1"
    res = run_bass_kernel_spmd(nc, in_maps, core_ids=list(range(M)), trace=trace)
    global LAST_RESULTS
    LAST_RESULTS = res

    acc = np.zeros(B, dtype=np.float64)
    wsum = 0.0
    for m in range(M):
        o = res.results[m]["out"].astype(np.float64)
        acc += o[:, :B].sum(axis=0)
        wsum += o[:, 8].sum()
    energies = acc - mu * wsum
    return energies.astype(np.float32)

